# revision 1
# baseline (speedup 1.0000x reference)
"""BiMamba (bidirectional Mamba block + LN + FFN) Trainium2 Bass kernel.

Sharding (8 cores): 4 scan-sequences (fwd/bwd x batch, bwd fed host-flipped x)
x 2 halves of d_inner. Device layout is feature-on-partitions /
time-on-free throughout; the host transposes x on the way in and the output
on the way out. Cross-core combines (out_proj partial sums + direction
merge, ff2 partial sums) use AllGather/AllReduce over quads
[0,1,4,5] / [2,3,6,7].
"""
import sys, os, types, contextlib, ctypes

sys.path.insert(0, "/opt/trn_rl_repo")
import numpy as np

D_MODEL = 1024
D_STATE = 16
D_CONV = 4
D_INNER = 2048
DT_RANK = 64
L = 1024
HALF = D_INNER // 2          # 1024 d_inner per core
P = 128
NJ = HALF // P               # 8 d-blocks per core half
TCH = 512                    # matmul t-chunk
NT = L // TCH
KD = D_MODEL // P            # 8 k-chunks over d_model
NFB_XC = D_INNER // P        # 16 xc feature blocks (full d_inner)
FF_SLICE = 1024              # ffn hidden slice per core
NB = DT_RANK + 2 * D_STATE   # 96

_GROUPS = [[0, 1, 4, 5], [2, 3, 6, 7]]


def _install_ntff_hook_shim(so_path="/opt/axon/libaxon_pjrt.so"):
    if "antenv.axon_hooks" in sys.modules:
        return
    try:
        lib = ctypes.CDLL(so_path)
    except OSError:
        return
    if not hasattr(lib, "axon_start_nrt_profile"):
        return
    lib.axon_start_nrt_profile.argtypes = [ctypes.POINTER(ctypes.c_int64), ctypes.c_size_t]
    lib.axon_start_nrt_profile.restype = ctypes.c_int64
    lib.axon_stop_nrt_profile.argtypes = [ctypes.c_char_p]
    lib.axon_stop_nrt_profile.restype = ctypes.c_int64

    @contextlib.contextmanager
    def _hook(output_dir, device_ids):
        import jax
        jax.devices()
        if device_ids:
            ids = (ctypes.c_int64 * len(device_ids))(*device_ids)
            rc = lib.axon_start_nrt_profile(ids, len(device_ids))
        else:
            rc = lib.axon_start_nrt_profile(None, 0)
        if rc != 0:
            raise RuntimeError(f"axon_start_nrt_profile rc={rc}")
        try:
            yield
        finally:
            n = lib.axon_stop_nrt_profile(str(output_dir).encode())
            print(f"profile: {n} file(s) written to {output_dir}", file=sys.stderr)

    mod = types.ModuleType("antenv.axon_hooks")
    mod.get_axon_ntff_profile_hook = lambda: _hook
    mod.set_axon_ntff_profile_hook = lambda h: None
    sys.modules["antenv.axon_hooks"] = mod


def _build_nc():
    from concourse import bacc, tile, mybir

    f32 = mybir.dt.float32
    f32r = mybir.dt.float32r
    bf16 = mybir.dt.bfloat16
    Alu = mybir.AluOpType
    Act = mybir.ActivationFunctionType

    def r(ap):
        return ap.bitcast(f32r)

    nc = bacc.Bacc("TRN2", target_bir_lowering=False, debug=False, num_devices=8)

    def din(name, shape, dt=None):
        return nc.dram_tensor(name, list(shape), dt or f32, kind="ExternalInput").ap()

    xT = din("xT", (D_MODEL, L), f32r)
    w_in_t = din("w_in_t", (NJ + NFB_XC, KD, P, P), f32r)          # z-half blocks, then xc
    convw_cols = din("convw_cols", (P, NFB_XC * D_CONV))
    convb_cols = din("convb_cols", (P, NFB_XC))
    xpw_t = din("xpw_t", (NFB_XC, P, P), f32r)  # cols: dt64|B16|pad|C16|pad
    dtw_t = din("dtw_t", (NJ, DT_RANK, P), f32r)
    dtb_cols = din("dtb_cols", (P, NJ))
    A_cols = din("A_cols", (P, NJ * D_STATE))
    D_colsT = din("D_colsT", (P, NJ))
    outw_t = din("outw_t", (KD, NJ, P, P), f32r)                   # [k(d_in), m(dm)]
    lng_cols = din("lng_cols", (P, KD))
    lnb_cols = din("lnb_cols", (P, KD))
    w1_t = din("w1_t", (KD, NJ, P, P), f32r)                       # [k(dm), m(h)]
    b1_cols = din("b1_cols", (P, NJ))
    w2_t = din("w2_t", (NJ, KD, P, P), f32r)                       # [k(h), m(dm)]
    b2_cols = din("b2_cols", (P, 2))
    consts_r = din("consts_r", (P, 4), f32r)  # col0=1/1024, cols1..3=0
    ident_r = din("ident_r", (P, P), f32r)

    out_m = nc.dram_tensor("out_m", [D_MODEL // 4, L], f32, kind="ExternalOutput").ap()

    es = contextlib.ExitStack()

    with tile.TileContext(nc) as tc:
        with contextlib.ExitStack() as stk:
            cpool = stk.enter_context(tc.tile_pool(name="cpool", bufs=1))
            psum = stk.enter_context(tc.tile_pool(name="psum", bufs=4, space="PSUM"))
            dram = stk.enter_context(tc.tile_pool(name="dram", bufs=1, space="DRAM"))

            def cload(src, shape, tag):
                t = cpool.tile(list(shape), f32, tag=tag, name=tag)
                nc.sync.dma_start(t[:], src)
                return t

            A_sb = cload(A_cols[:], (P, NJ * D_STATE), "A_sb")
            dtb_sb = cload(dtb_cols[:], (P, NJ), "dtb_sb")
            D_sb = cload(D_colsT[:], (P, NJ), "D_sb")
            convb_sb = cload(convb_cols[:], (P, NFB_XC), "convb_sb")
            convw_sb = cload(convw_cols[:], (P, NFB_XC * D_CONV), "convw_sb")
            lng_sb = cload(lng_cols[:], (P, KD), "lng_sb")
            lnb_sb = cload(lnb_cols[:], (P, KD), "lnb_sb")
            b1_sb = cload(b1_cols[:], (P, NJ), "b1_sb")
            b2_sb = cload(b2_cols[:], (P, 2), "b2_sb")
            ones_sb = cpool.tile([P, 1], f32r, tag="ones_sb", name="ones_sb")
            nc.sync.dma_start(ones_sb[:], consts_r[:, 0:1])
            ident_sb = cpool.tile([P, P], f32r, tag="ident_sb", name="ident_sb")
            nc.sync.dma_start(ident_sb[:], ident_r[:])

            bcB_dram = dram.tile([D_STATE, L], bf16, name="bcB_dram")
            bcC_dram = dram.tile([D_STATE, L], f32, name="bcC_dram")
            stat_dram = dram.tile([2, L], f32, name="stat_dram")
            HD = D_MODEL // 2
            ag_in = [dram.tile([HD, L], f32, name=f"ag_in{h}") for h in range(2)]
            ag_out = [dram.tile([4 * HD, L], f32, name=f"ag_out{h}") for h in range(2)]
            ar_in = dram.tile([D_MODEL, L], f32, name="ar_in")
            rs_out = dram.tile([D_MODEL // 4, L], f32, name="rs_out")

            def mm_accum(ps, lw_list, rhs_of_k, n_k):
                for k in range(n_k):
                    nc.tensor.matmul(ps[:], lw_list[k][:], rhs_of_k(k),
                                     start=(k == 0), stop=(k == n_k - 1))

            # =========== P1-P4 region: sz lives until gating ===========
            with tc.tile_pool(name="sz_pool", bufs=1) as sz_pool:
                sz = [sz_pool.tile([P, L], f32, tag=f"sz{j}", name=f"sz{j}")
                      for j in range(NJ)]
                dt_sb = sz_pool.tile([DT_RANK, L], f32r, tag="dt_sb", name="dt_sb")

                def dt_proj_delta(j, delta_t, pool):
                    # softplus(x + b) = Ln(1 + Exp(x + b)); inputs here are
                    # well below 0 so Exp cannot overflow.
                    lw = pool.tile([DT_RANK, P], f32r, tag="dtw", name=f"dtw{j}", bufs=2)
                    nc.sync.dma_start(lw[:], dtw_t[j])
                    for t in range(NT):
                        ps = psum.tile([P, TCH], f32, tag="ps", name=f"dtp{j}_{t}")
                        nc.tensor.matmul(ps[:], lw[:],
                                         dt_sb[:, t * TCH:(t + 1) * TCH],
                                         start=True, stop=True)
                        spt = pool.tile([P, TCH], f32, tag="spt", name=f"spt{j}_{t}",
                                        bufs=2)
                        nc.scalar.activation(spt[:], ps[:], Act.Exp,
                                             bias=dtb_sb[:, j:j + 1])
                        nc.scalar.activation(delta_t[:, t * TCH:(t + 1) * TCH], spt[:],
                                             Act.Ln, bias=1.0)

                # ---------------- P1..P3: need xc blocks ----------------
                with tc.tile_pool(name="xc_pool", bufs=1) as xc_pool:
                    xcs = [xc_pool.tile([P, L], f32r, tag=f"xcs{j}", name=f"xcs{j}")
                           for j in range(NFB_XC)]

                    # P1: in_proj + conv + silu
                    with tc.tile_pool(name="xt_pool", bufs=1) as xt_pool, \
                         tc.tile_pool(name="p1t", bufs=1) as p1t:
                        xts = []
                        for k in range(KD):
                            xt_k = xt_pool.tile([P, L], f32r, tag=f"xt{k}", name=f"xt{k}")
                            nc.sync.dma_start(xt_k[:], xT[k * P:(k + 1) * P, :])
                            xts.append(xt_k)

                        def in_proj_block(fb):
                            lws = []
                            for k in range(KD):
                                lw = p1t.tile([P, P], f32r, tag=f"lw{k}",
                                              name=f"lw{fb}_{k}", bufs=2)
                                nc.sync.dma_start(lw[:], w_in_t[fb, k])
                                lws.append(lw)
                            pss = []
                            for t in range(NT):
                                ps = psum.tile([P, TCH], f32, tag="ps",
                                               name=f"inp{fb}_{t}")
                                mm_accum(ps, lws,
                                         lambda k: xts[k][:, t * TCH:(t + 1) * TCH], KD)
                                pss.append(ps)
                            return pss

                        for j in range(NJ):  # z half
                            for t, ps in enumerate(in_proj_block(j)):
                                nc.scalar.activation(sz[j][:, t * TCH:(t + 1) * TCH],
                                                     ps[:], Act.Silu)

                        for j in range(NFB_XC):  # xc blocks + conv (DVE taps)
                            xcp = p1t.tile([P, L + D_CONV - 1], f32, tag="xcp",
                                           name=f"xcp{j}", bufs=2)
                            nc.sync.dma_start(xcp[:, 0:D_CONV - 1],
                                              consts_r[:, 1:D_CONV].bitcast(f32))
                            for t, ps in enumerate(in_proj_block(NJ + j)):
                                nc.scalar.copy(
                                    xcp[:, D_CONV - 1 + t * TCH:D_CONV - 1 + (t + 1) * TCH],
                                    ps[:])
                            cacc = p1t.tile([P, L], f32, tag="cacc",
                                            name=f"cacc{j}", bufs=2)
                            nc.vector.tensor_scalar_mul(
                                cacc[:], xcp[:, 0:L],
                                convw_sb[:, j * D_CONV:j * D_CONV + 1])
                            for i in range(1, D_CONV):
                                nc.vector.scalar_tensor_tensor(
                                    cacc[:], xcp[:, i:i + L],
                                    convw_sb[:, j * D_CONV + i:j * D_CONV + i + 1],
                                    cacc[:], Alu.mult, Alu.add)
                            nc.scalar.activation(xcs[j][:], cacc[:], Act.Silu,
                                                 bias=convb_sb[:, j:j + 1])

                    # right-side pools for wv/g0 (live P2..P4)
                    wvg_ctx = contextlib.ExitStack()
                    wv_pool = wvg_ctx.enter_context(
                        tc.tile_pool(name="wv_pool", bufs=1, side="right"))
                    wvs = [wv_pool.tile([P, L], bf16, tag=f"wv{j}", name=f"wv{j}")
                           for j in range(NJ)]
                    g0_pool = wvg_ctx.enter_context(
                        tc.tile_pool(name="g0_pool", bufs=1, side="right"))
                    g0s = [g0_pool.tile([P, L], f32, tag=f"g0{j}", name=f"g0{j}")
                           for j in range(NJ)]

                    # P2: x_proj; P3: wv/g0
                    with tc.tile_pool(name="p2t", bufs=1) as p2t:
                        for t in range(NT):
                            ps = psum.tile([P, TCH], f32, tag="ps", name=f"xproj{t}")
                            for k in range(NFB_XC):
                                lw = p2t.tile([P, P], f32r, tag="xpw",
                                              name=f"xpw{t}_{k}", bufs=2)
                                nc.sync.dma_start(lw[:], xpw_t[k])
                                nc.tensor.matmul(ps[:], lw[:],
                                                 xcs[k][:, t * TCH:(t + 1) * TCH],
                                                 start=(k == 0), stop=(k == NFB_XC - 1))
                            nc.scalar.copy(dt_sb[:, t * TCH:(t + 1) * TCH],
                                           ps[0:DT_RANK, :])
                            bcB_sb = p2t.tile([D_STATE, TCH], bf16, tag="bcB_sb",
                                              name=f"bcB_sb{t}", bufs=2)
                            nc.scalar.copy(bcB_sb[:], ps[64:80, :])
                            nc.sync.dma_start(bcB_dram[:, t * TCH:(t + 1) * TCH], bcB_sb[:])
                            bcC_sb = p2t.tile([D_STATE, TCH], f32, tag="bcC_sb",
                                              name=f"bcC_sb{t}", bufs=2)
                            nc.scalar.copy(bcC_sb[:], ps[96:112, :])
                            nc.sync.dma_start(bcC_dram[:, t * TCH:(t + 1) * TCH], bcC_sb[:])

                        for j in range(NJ):
                            dtmp = p2t.tile([P, L], f32, tag="dtmp", name=f"dtmp{j}",
                                            bufs=2)
                            dt_proj_delta(j, dtmp, p2t)
                            nc.vector.tensor_tensor(wvs[j][:], dtmp[:], xcs[j][:],
                                                    Alu.mult)
                            t1 = p2t.tile([P, L], f32, tag="g0tmp", name=f"g0tmp{j}",
                                          bufs=2)
                            nc.vector.tensor_scalar_mul(t1[:], xcs[j][:], D_sb[:, j:j + 1])
                            nc.vector.tensor_tensor(g0s[j][:], t1[:], sz[j][:], Alu.mult)

                # ---------------- P4: scan + gating ----------------
                yg_ctx = contextlib.ExitStack()
                yg_pool = yg_ctx.enter_context(tc.tile_pool(name="yg_pool", bufs=1))
                ygs = [yg_pool.tile([P, L], f32r, tag=f"yg{j}", name=f"yg{j}")
                       for j in range(NJ)]
                with tc.tile_pool(name="spool", bufs=1) as spool, \
                     tc.tile_pool(name="tpool", bufs=1) as tpool, \
                     tc.tile_pool(name="pscan", bufs=1, space="PSUM") as pscan:
                    for hb in range(4):
                        js = list(range(hb * 2, hb * 2 + 2))
                        deltas = {}
                        yps = {}
                        for j in js:
                            dj = spool.tile([P, L], f32, tag=f"delta{j % 2}",
                                            name=f"delta{j}")
                            dt_proj_delta(j, dj, spool)
                            deltas[j] = dj
                            yps[j] = pscan.tile([P, L], f32, tag=f"yps{j % 2}",
                                                name=f"yps{j}")
                        for n in range(D_STATE):
                            Bbc = tpool.tile([P, L], bf16, tag="Bbc",
                                             name=f"Bbc{hb}_{n}", bufs=2)
                            nc.sync.dma_start(
                                Bbc[:],
                                bcB_dram[n:n + 1, :].partition_broadcast(P).squeeze(1))
                            Cbc = tpool.tile([P, L], f32, tag="Cbc",
                                             name=f"Cbc{hb}_{n}", bufs=2)
                            nc.sync.dma_start(
                                Cbc[:],
                                bcC_dram[n:n + 1, :].partition_broadcast(P).squeeze(1))
                            for j in js:
                                a_t = tpool.tile([P, L], f32, tag="a_t",
                                                 name=f"a{j}_{n}", bufs=3)
                                nc.scalar.activation(
                                    a_t[:], deltas[j][:], Act.Exp,
                                    scale=A_sb[:, j * D_STATE + n:j * D_STATE + n + 1])
                                b_t = tpool.tile([P, L], bf16, tag="b_t",
                                                 name=f"b{j}_{n}", bufs=1)
                                nc.vector.tensor_tensor(b_t[:], wvs[j][:], Bbc[:],
                                                        Alu.mult)
                                h_t = tpool.tile([P, L], f32, tag="h_t",
                                                 name=f"h{j}_{n}", bufs=2)
                                nc.vector.tensor_tensor_scan(h_t[:], a_t[:], b_t[:],
                                                             0.0, Alu.mult, Alu.add)
                                prod = tpool.tile([P, L], f32r, tag="prod",
                                                  name=f"p{j}_{n}", bufs=3)
                                nc.vector.tensor_tensor(prod[:], h_t[:], Cbc[:],
                                                        Alu.mult)
                                for t in range(NT):
                                    sl = slice(t * TCH, (t + 1) * TCH)
                                    nc.tensor.matmul(yps[j][:, sl], ident_sb[:],
                                                     prod[:, sl],
                                                     start=(n == 0),
                                                     stop=(n == D_STATE - 1))
                        for j in js:
                            ygt = tpool.tile([P, L], f32, tag="ygt",
                                             name=f"ygt{j}", bufs=2)
                            nc.vector.tensor_tensor(ygt[:], yps[j][:], sz[j][:],
                                                    Alu.mult)
                            nc.vector.tensor_tensor(ygs[j][:], ygt[:], g0s[j][:],
                                                    Alu.add)
                wvg_ctx.close()  # wv + g0 (right side, LIFO: g0 then wv)

                # =========== P5: out_proj partials + split AllGather ===========
                with tc.tile_pool(name="p5t", bufs=1) as p5t:
                    for m in range(NJ):
                        lws = []
                        for k in range(KD):
                            lw = p5t.tile([P, P], f32r, tag=f"lw{k}",
                                          name=f"ow{m}_{k}", bufs=2)
                            nc.sync.dma_start(lw[:], outw_t[k, m])
                            lws.append(lw)
                        msb = p5t.tile([P, L], f32, tag="msb", name=f"msb{m}", bufs=2)
                        for t in range(NT):
                            ps = psum.tile([P, TCH], f32, tag="ps", name=f"op{m}_{t}")
                            mm_accum(ps, lws,
                                     lambda k: ygs[k][:, t * TCH:(t + 1) * TCH], KD)
                            nc.scalar.copy(msb[:, t * TCH:(t + 1) * TCH], ps[:])
                        h = m // 4
                        nc.sync.dma_start(
                            ag_in[h][(m % 4) * P:(m % 4 + 1) * P, :], msb[:])
                        if m == 3 or m == NJ - 1:
                            nc.gpsimd.collective_compute(
                                "AllGather", Alu.bypass, replica_groups=_GROUPS,
                                ins=[ag_in[h][:]], outs=[ag_out[h][:]])
                yg_ctx.close()

            # =========== P6: mo + LN ===========
            with contextlib.ExitStack() as stk2:
                mo_pool = stk2.enter_context(tc.tile_pool(name="mo_pool", bufs=1))
                mos = [mo_pool.tile([P, L], f32r, tag=f"mo{j}", name=f"mo{j}")
                       for j in range(KD)]
                xn_pool = stk2.enter_context(tc.tile_pool(name="xn_pool", bufs=1))
                xns = [xn_pool.tile([P, L], f32r, tag=f"xn{j}", name=f"xn{j}")
                       for j in range(KD)]

                with tc.tile_pool(name="p6t", bufs=1) as p6t, \
                     tc.tile_pool(name="pln", bufs=1, space="PSUM") as pln:
                    mu_ps = pln.tile([1, L], f32, tag="mu_ps", name="mu_ps", bufs=1)
                    e2_ps = pln.tile([1, L], f32, tag="e2_ps", name="e2_ps", bufs=1)
                    for j in range(KD):
                        parts = []
                        h, jm = j // 4, j % 4
                        for q in range(4):
                            pt = p6t.tile([P, L], f32, tag="agp", name=f"agp{j}_{q}",
                                          bufs=4)
                            nc.sync.dma_start(
                                pt[:],
                                ag_out[h][q * 4 * P + jm * P:q * 4 * P + (jm + 1) * P, :])
                            parts.append(pt)
                        a01 = p6t.tile([P, L], f32, tag="a01", name=f"a01_{j}", bufs=2)
                        nc.vector.tensor_tensor(a01[:], parts[0][:], parts[1][:],
                                                Alu.add)
                        a23 = p6t.tile([P, L], f32, tag="a23", name=f"a23_{j}", bufs=2)
                        nc.vector.tensor_tensor(a23[:], parts[2][:], parts[3][:],
                                                Alu.add)
                        nc.vector.tensor_tensor(mos[j][:], a01[:], a23[:, ::-1],
                                                Alu.add)
                        sq = p6t.tile([P, L], f32r, tag="sq", name=f"sq{j}", bufs=2)
                        nc.scalar.activation(sq[:], mos[j][:], Act.Square)
                        for t in range(NT):
                            sl = slice(t * TCH, (t + 1) * TCH)
                            nc.tensor.matmul(mu_ps[:, sl], ones_sb[:],
                                             mos[j][:, sl],
                                             start=(j == 0), stop=(j == KD - 1))
                            nc.tensor.matmul(e2_ps[:, sl], ones_sb[:],
                                             sq[:, sl],
                                             start=(j == 0), stop=(j == KD - 1))

                    mean_sb = p6t.tile([1, L], f32r, tag="mean_sb", name="mean_sb",
                                       bufs=1)
                    nc.scalar.copy(mean_sb[:], mu_ps[:])
                    m2 = p6t.tile([1, L], f32, tag="m2", name="m2", bufs=1)
                    nc.vector.tensor_tensor(m2[:], mean_sb[:], mean_sb[:], Alu.mult)
                    var_t = p6t.tile([1, L], f32, tag="var_t", name="var_t", bufs=1)
                    nc.vector.tensor_tensor(var_t[:], e2_ps[:], m2[:], Alu.subtract)
                    eps_sb = p6t.tile([1, 1], f32, tag="eps_sb", name="eps_sb", bufs=1)
                    nc.vector.memset(eps_sb[:], 1e-5)
                    std_t = p6t.tile([1, L], f32, tag="std_t", name="std_t", bufs=1)
                    nc.scalar.activation(std_t[:], var_t[:], Act.Sqrt, bias=eps_sb[:])
                    rstd_sb = p6t.tile([1, L], f32r, tag="rstd_sb", name="rstd_sb",
                                       bufs=1)
                    with nc.allow_low_precision(reason="f32r view of fp32 rstd"):
                        nc.vector.reciprocal(rstd_sb[:], std_t[:])
                    onesrow = p6t.tile([1, P], f32r, tag="onesrow", name="onesrow",
                                       bufs=1)
                    nc.scalar.activation(onesrow[:], ident_sb[0:1, :], Act.Copy,
                                         bias=1.0, scale=0.0)
                    mean_bc = pln.tile([P, L], f32, tag="mu_ps", name="mean_bc",
                                       bufs=1)
                    rstd_bc = pln.tile([P, L], f32, tag="e2_ps", name="rstd_bc",
                                       bufs=1)
                    for t in range(NT):
                        sl = slice(t * TCH, (t + 1) * TCH)
                        nc.tensor.matmul(mean_bc[:, sl], onesrow[:], mean_sb[:, sl],
                                         start=True, stop=True)
                        nc.tensor.matmul(rstd_bc[:, sl], onesrow[:], rstd_sb[:, sl],
                                         start=True, stop=True)

                    for j in range(KD):
                        t1 = p6t.tile([P, L], f32, tag="lnt", name=f"lnt{j}", bufs=2)
                        nc.vector.tensor_tensor(t1[:], mos[j][:], mean_bc[:],
                                                Alu.subtract)
                        nc.vector.tensor_tensor(t1[:], t1[:], rstd_bc[:], Alu.mult)
                        nc.vector.tensor_scalar(xns[j][:], t1[:], lng_sb[:, j:j + 1],
                                                lnb_sb[:, j:j + 1], Alu.mult, Alu.add)

                # =========== P7: FFN ===========
                with tc.tile_pool(name="ffh_pool", bufs=1) as ffh_pool, \
                     tc.tile_pool(name="p7t", bufs=1) as p7t:
                    ffhs = [ffh_pool.tile([P, L], f32r, tag=f"ffh{m}", name=f"ffh{m}")
                            for m in range(NJ)]
                    for m in range(NJ):
                        lws = []
                        for k in range(KD):
                            lw = p7t.tile([P, P], f32r, tag=f"lw{k}", name=f"w1_{m}_{k}",
                                          bufs=2)
                            nc.sync.dma_start(lw[:], w1_t[k, m])
                            lws.append(lw)
                        for t in range(NT):
                            ps = psum.tile([P, TCH], f32, tag="ps", name=f"f1{m}_{t}")
                            mm_accum(ps, lws,
                                     lambda k: xns[k][:, t * TCH:(t + 1) * TCH], KD)
                            nc.scalar.activation(ffhs[m][:, t * TCH:(t + 1) * TCH],
                                                 ps[:], Act.Gelu,
                                                 bias=b1_sb[:, m:m + 1])

                    for m in range(KD):
                        lws = []
                        for k in range(NJ):
                            lw = p7t.tile([P, P], f32r, tag=f"lw{k}", name=f"w2_{m}_{k}",
                                          bufs=2)
                            nc.sync.dma_start(lw[:], w2_t[k, m])
                            lws.append(lw)
                        msb = p7t.tile([P, L], f32, tag="msb", name=f"f2sb{m}", bufs=2)
                        for t in range(NT):
                            ps = psum.tile([P, TCH], f32, tag="ps", name=f"f2{m}_{t}")
                            mm_accum(ps, lws,
                                     lambda k: ffhs[k][:, t * TCH:(t + 1) * TCH], NJ)
                            nc.scalar.copy(msb[:, t * TCH:(t + 1) * TCH], ps[:])
                        nc.sync.dma_start(ar_in[m * P:(m + 1) * P, :], msb[:])

            nc.gpsimd.collective_compute("ReduceScatter", Alu.add,
                                         replica_groups=_GROUPS,
                                         ins=[ar_in[:]], outs=[rs_out[:]])

            with tc.tile_pool(name="p8t", bufs=1) as p8t:
                for j in range(2):
                    fin = p8t.tile([P, L], f32, tag="fin", name=f"fin{j}", bufs=2)
                    nc.sync.dma_start(fin[:], rs_out[j * P:(j + 1) * P, :])
                    fob = p8t.tile([P, L], f32, tag="fob", name=f"fob{j}", bufs=2)
                    nc.vector.tensor_scalar_add(fob[:], fin[:], b2_sb[:, j:j + 1])
                    nc.sync.dma_start(out_m[j * P:(j + 1) * P, :], fob[:])

    nc.compile()
    return nc


def _prep_inputs(inputs):
    """Per-core input dicts. Core c: sequence s=c//2 (s>=2 => time-flipped x),
    d_inner half = c%2. The own half of d_inner is permuted FIRST in every
    d_inner-ordered tensor, so the device kernel is identical on all cores."""
    x = np.asarray(inputs["x"], dtype=np.float32)
    in_proj_w = np.asarray(inputs["in_proj_w"], dtype=np.float32)
    conv_w = np.asarray(inputs["conv_w"], dtype=np.float32)
    conv_b = np.asarray(inputs["conv_b"], dtype=np.float32)
    x_proj_w = np.asarray(inputs["x_proj_w"], dtype=np.float32)
    dt_proj_w = np.asarray(inputs["dt_proj_w"], dtype=np.float32)
    dt_proj_b = np.asarray(inputs["dt_proj_b"], dtype=np.float32)
    A = -np.exp(np.asarray(inputs["A_log"], dtype=np.float32))
    Dp = np.asarray(inputs["D"], dtype=np.float32)
    out_proj_w = np.asarray(inputs["out_proj_w"], dtype=np.float32)
    ln_g = np.asarray(inputs["ln_g"], dtype=np.float32)
    ln_b = np.asarray(inputs["ln_b"], dtype=np.float32)
    ff_w1 = np.asarray(inputs["ff_w1"], dtype=np.float32)
    ff_b1 = np.asarray(inputs["ff_b1"], dtype=np.float32)
    ff_w2 = np.asarray(inputs["ff_w2"], dtype=np.float32)
    ff_b2 = np.asarray(inputs["ff_b2"], dtype=np.float32)

    def cols(v):  # (N,) -> (P, N//P) per-partition column layout
        return np.ascontiguousarray(v.reshape(-1, P).T)

    def tile_w(w, KP, MP):  # (K, M) -> (K//KP, M//MP, KP, MP)
        K, M = w.shape
        return np.ascontiguousarray(
            w.reshape(K // KP, KP, M // MP, MP).transpose(0, 2, 1, 3))

    in_maps = []
    for c in range(8):
        s, half = c // 2, c % 2
        xb = x[s] if s < 2 else x[s - 2][::-1]
        perm = np.arange(D_INNER).reshape(2, HALF)
        perm = np.concatenate([perm[half], perm[1 - half]])
        own = perm[:HALF]

        wz = in_proj_w[:, D_INNER + own]                      # (1024, 1024)
        wxc = in_proj_w[:, perm]                              # (1024, 2048)
        w_in = np.concatenate([wz, wxc], axis=1)              # (1024, 3072)
        w_in_t = np.ascontiguousarray(tile_w(w_in, P, P).transpose(1, 0, 2, 3))

        cw = conv_w[perm]  # (2048, 4) -> (P, 16*4): col j*4+i = w[jP+p, i]
        convw_cols = np.ascontiguousarray(
            cw.reshape(NFB_XC, P, D_CONV).transpose(1, 0, 2).reshape(P, NFB_XC * D_CONV))

        g = (c & 1) + 2 * (c >> 2)
        hsl = slice(g * FF_SLICE, (g + 1) * FF_SLICE)

        in_maps.append({
            "xT": np.ascontiguousarray(xb.T),
            "w_in_t": w_in_t,
            "convw_cols": convw_cols,
            "convb_cols": cols(conv_b[perm]),
            "xpw_t": np.ascontiguousarray(
                np.concatenate([
                    x_proj_w[perm][:, :DT_RANK + D_STATE],
                    np.zeros((D_INNER, D_STATE), np.float32),
                    x_proj_w[perm][:, DT_RANK + D_STATE:],
                    np.zeros((D_INNER, D_STATE), np.float32),
                ], axis=1).reshape(NFB_XC, P, P)),
            "dtw_t": np.ascontiguousarray(
                dt_proj_w[:, own].reshape(DT_RANK, NJ, P).transpose(1, 0, 2)),
            "dtb_cols": cols(dt_proj_b[own]),
            "A_cols": np.ascontiguousarray(
                A[own].reshape(NJ, P, D_STATE).transpose(1, 0, 2).reshape(P, NJ * D_STATE)),
            "D_colsT": cols(Dp[own]),
            "outw_t": tile_w(out_proj_w[own], P, P),
            "lng_cols": cols(ln_g),
            "lnb_cols": cols(ln_b),
            "w1_t": tile_w(ff_w1[:, hsl], P, P),
            "b1_cols": cols(ff_b1[hsl]),
            "w2_t": tile_w(ff_w2[hsl], P, P),
            "b2_cols": cols(ff_b2[g * 256:(g + 1) * 256]),
            "ident_r": np.eye(P, dtype=np.float32),
            "consts_r": np.concatenate(
                [np.full((P, 1), 1.0 / D_MODEL, np.float32),
                 np.zeros((P, 3), np.float32)], axis=1),
        })
    return in_maps


_NC_CACHE = {}


def _get_nc():
    if "nc" not in _NC_CACHE:
        _NC_CACHE["nc"] = _build_nc()
    return _NC_CACHE["nc"]


def run(inputs, trace=False):
    _install_ntff_hook_shim()
    from concourse import bass_utils
    nc = _get_nc()
    in_maps = _prep_inputs(inputs)
    res = bass_utils.run_bass_kernel_spmd(nc, in_maps, core_ids=list(range(8)),
                                          trace=trace)
    # each core holds the dm-quarter (rows g*256..) of its group's output
    full = np.zeros((2, D_MODEL, L), np.float32)
    for c in range(8):
        b = 0 if c in (0, 1, 4, 5) else 1
        g = (c & 1) + 2 * (c >> 2)
        full[b, g * 256:(g + 1) * 256, :] = res.results[c]["out_m"]
    out = np.ascontiguousarray(full.transpose(0, 2, 1))
    return out, res


def kernel(**inputs):
    out, _ = run(inputs, trace=False)
    return out



# revision 8
# speedup vs baseline: 1.8173x; 1.8173x over previous
"""BiMamba (bidirectional Mamba block + LN + FFN) Trainium2 Bass kernel.

Sharding (8 cores): 4 scan-sequences (fwd/bwd x batch, bwd fed host-flipped x)
x 2 halves of d_inner. Feature-on-partitions / time-on-free throughout.

Redesign vs baseline:
- Each core computes in_proj only for its own d_inner half (xc own + z own);
  the x_proj contraction over the full d_inner is completed with a pair
  AllReduce of the (128, L) x_proj partial sums.
- All large GEMMs run in bf16/fp16 (1 cycle/row + fast FWL weight loads).
- Scan phase all fp16: exp on Act engine, b/prod multiplies on DVE at the
  2x packed rate, tensor_tensor_scan fp16, state-sum via fp16 identity
  matmuls into PSUM.
- Direction merge + FFN input distribution via ONE ReduceScatter over quads
  that scatters along TIME: each core then owns a 256-column t-slice, does
  LN + the full FFN locally (weights streamed JIT), no further collectives.
  The bwd-core time flip is handled with per-core 0/1 flag columns scaling
  a straight and a reversed copy into separate RS slots (identical program
  on all cores).
"""
import sys, os, types, contextlib, ctypes

sys.path.insert(0, "/opt/trn_rl_repo")
import numpy as np

D_MODEL = 1024
D_STATE = 16
D_CONV = 4
D_INNER = 2048
DT_RANK = 64
L = 1024
HALF = D_INNER // 2          # 1024 d_inner per core
P = 128
NJ = HALF // P               # 8 d-blocks per core half
TCH = 512                    # matmul t-chunk
NT = L // TCH
KD = D_MODEL // P            # 8 k-chunks over d_model
NH1 = 4 * D_MODEL // P       # 32 ffn hidden blocks
LQ = L // 4                  # 256 t-slice per core after RS

_PAIRS = [[0, 1], [2, 3], [4, 5], [6, 7]]
_QUADS = [[0, 1, 4, 5], [2, 3, 6, 7]]
GP_N = ()


def _install_ntff_hook_shim(so_path="/opt/axon/libaxon_pjrt.so"):
    if "antenv.axon_hooks" in sys.modules:
        return
    try:
        lib = ctypes.CDLL(so_path)
    except OSError:
        return
    if not hasattr(lib, "axon_start_nrt_profile"):
        return
    lib.axon_start_nrt_profile.argtypes = [ctypes.POINTER(ctypes.c_int64), ctypes.c_size_t]
    lib.axon_start_nrt_profile.restype = ctypes.c_int64
    lib.axon_stop_nrt_profile.argtypes = [ctypes.c_char_p]
    lib.axon_stop_nrt_profile.restype = ctypes.c_int64

    @contextlib.contextmanager
    def _hook(output_dir, device_ids):
        import jax
        jax.devices()
        if device_ids:
            ids = (ctypes.c_int64 * len(device_ids))(*device_ids)
            rc = lib.axon_start_nrt_profile(ids, len(device_ids))
        else:
            rc = lib.axon_start_nrt_profile(None, 0)
        if rc != 0:
            raise RuntimeError(f"axon_start_nrt_profile rc={rc}")
        try:
            yield
        finally:
            n = lib.axon_stop_nrt_profile(str(output_dir).encode())
            print(f"profile: {n} file(s) written to {output_dir}", file=sys.stderr)

    mod = types.ModuleType("antenv.axon_hooks")
    mod.get_axon_ntff_profile_hook = lambda: _hook
    mod.set_axon_ntff_profile_hook = lambda h: None
    sys.modules["antenv.axon_hooks"] = mod


def _build_nc():
    from concourse import bacc, tile, mybir

    f32 = mybir.dt.float32
    bf16 = mybir.dt.bfloat16
    fp16 = mybir.dt.float16
    Alu = mybir.AluOpType
    Act = mybir.ActivationFunctionType

    nc = bacc.Bacc("TRN2", target_bir_lowering=False, debug=False, num_devices=8)

    def din(name, shape, dt):
        return nc.dram_tensor(name, list(shape), dt, kind="ExternalInput").ap()

    xT = din("xT", (D_MODEL, L), bf16)
    w_in = din("w_in", (16, P, KD, P), bf16)        # fb 0..7 xc-own, 8..15 z-own
    convw_cols = din("convw_cols", (P, NJ * D_CONV), f32)
    convb_cols = din("convb_cols", (P, NJ), f32)
    xpw = din("xpw", (P, NJ, P), fp16)              # [p, k, n] n: dt64|B16|C16|pad
    dtw = din("dtw", (DT_RANK, NJ, P), fp16)
    dtb_cols = din("dtb_cols", (P, NJ), f32)
    A_cols = din("A_cols", (P, NJ * D_STATE), f32)
    D_cols = din("D_cols", (P, NJ), f32)
    outw = din("outw", (NJ, P, NJ, P), fp16)        # [m, p(k-part), k, mp]
    lng_cols = din("lng_cols", (P, KD), f32)
    lnb_cols = din("lnb_cols", (P, KD), f32)
    w1m = din("w1m", (NH1, P, KD, P), bf16)
    b1_cols = din("b1_cols", (P, NH1), f32)
    w2m = din("w2m", (KD, P, NH1, P), bf16)
    b2_cols = din("b2_cols", (P, KD), f32)
    identh = din("identh", (P, P), fp16)
    ones_h = din("ones_h", (P, 2), fp16)            # col0: ones (stats lhsT)
    onesrow_h = din("onesrow_h", (1, P), fp16)      # bcast lhsT
    flags = din("flags", (P, 2), f32)               # col0 fwd, col1 bwd

    out_m = nc.dram_tensor("out_m", [D_MODEL, LQ], f32, kind="ExternalOutput").ap()

    with tile.TileContext(nc) as tc:
        with contextlib.ExitStack() as stk:
            cpool = stk.enter_context(tc.tile_pool(name="cpool", bufs=1))
            dram = stk.enter_context(tc.tile_pool(name="dram", bufs=1, space="DRAM"))

            def cload(src, shape, dt, tag):
                t = cpool.tile(list(shape), dt, tag=tag, name=tag)
                nc.sync.dma_start(t[:], src)
                return t

            # input activations first in the DMA queue
            xts = []
            for k in range(KD):
                xt_k = cpool.tile([P, L], bf16, tag=f"xt{k}", name=f"xt{k}")
                nc.sync.dma_start(xt_k[:], xT[k * P:(k + 1) * P, :])
                xts.append(xt_k)

            convw_sb = cload(convw_cols[:], (P, NJ * D_CONV), f32, "convw_sb")
            convb_sb = cload(convb_cols[:], (P, NJ), f32, "convb_sb")
            dtb_sb = cload(dtb_cols[:], (P, NJ), f32, "dtb_sb")
            A_sb = cload(A_cols[:], (P, NJ * D_STATE), f32, "A_sb")
            D_sb = cload(D_cols[:], (P, NJ), f32, "D_sb")
            lng_sb = cload(lng_cols[:], (P, KD), f32, "lng_sb")
            lnb_sb = cload(lnb_cols[:], (P, KD), f32, "lnb_sb")
            b1_sb = cload(b1_cols[:], (P, NH1), f32, "b1_sb")
            b2_sb = cload(b2_cols[:], (P, KD), f32, "b2_sb")
            ident_sb = cload(identh[:], (P, P), fp16, "ident_sb")
            ones_sb = cload(ones_h[:], (P, 2), fp16, "ones_sb")
            onesrow_sb = cload(onesrow_h[:], (1, P), fp16, "onesrow_sb")
            flags_sb = cload(flags[:], (P, 2), f32, "flags_sb")
            xpw_sb = cload(xpw[:], (P, NJ * P), fp16, "xpw_sb")
            dtw_sb = cload(dtw[:], (DT_RANK, NJ * P), fp16, "dtw_sb")

            dbl_in = dram.tile([P, L], fp16, name="dbl_in")
            dbl_out = dram.tile([P, L], fp16, name="dbl_out")
            bcBC = dram.tile([D_STATE, 2 * L], fp16, name="bcBC")
            arqA = dram.tile([4, D_MODEL // 2, LQ], fp16, name="arqA")
            arqB = dram.tile([4, D_MODEL // 2, LQ], fp16, name="arqB")
            rs_outA = dram.tile([D_MODEL // 2, LQ], fp16, name="rs_outA")
            rs_outB = dram.tile([D_MODEL // 2, LQ], fp16, name="rs_outB")

            # persistent SBUF (P1->P4/P5)
            sz_pool = stk.enter_context(tc.tile_pool(name="sz_pool", bufs=1))
            szs = [sz_pool.tile([P, L], fp16, tag=f"sz{j}", name=f"sz{j}")
                   for j in range(NJ)]
            dl_pool = stk.enter_context(tc.tile_pool(name="dl_pool", bufs=1))
            deltas = [dl_pool.tile([P, L], fp16, tag=f"dl{j}", name=f"dl{j}")
                      for j in range(NJ)]
            wv_pool = stk.enter_context(tc.tile_pool(name="wv_pool", bufs=1))
            wvs = [wv_pool.tile([P, L], fp16, tag=f"wv{j}", name=f"wv{j}")
                   for j in range(NJ)]
            g0_pool = stk.enter_context(tc.tile_pool(name="g0_pool", bufs=1))
            g0s = [g0_pool.tile([P, L], fp16, tag=f"g0{j}", name=f"g0{j}")
                   for j in range(NJ)]
            yg_pool = stk.enter_context(tc.tile_pool(name="yg_pool", bufs=1))
            ygs = [yg_pool.tile([P, L], fp16, tag=f"yg{j}", name=f"yg{j}")
                   for j in range(NJ)]

            # ================= P1-P3 =================
            with tc.tile_pool(name="xc_pool", bufs=1) as xc_pool, \
                 tc.tile_pool(name="p13", bufs=1) as p13, \
                 tc.tile_pool(name="psA", bufs=4, space="PSUM") as psA:
                xcs = [xc_pool.tile([P, L], fp16, tag=f"xc{j}", name=f"xc{j}")
                       for j in range(NJ)]

                def in_proj_block(fb, tag):
                    lw = p13.tile([P, KD * P], bf16, tag=tag, name=f"{tag}_{fb}",
                                  bufs=2)
                    nc.sync.dma_start(lw[:], w_in[fb])
                    pss = []
                    for t in range(NT):
                        ps = psA.tile([P, TCH], f32, tag="ps", name=f"inp{fb}_{t}")
                        for k in range(KD):
                            nc.tensor.matmul(ps[:], lw[:, k * P:(k + 1) * P],
                                             xts[k][:, t * TCH:(t + 1) * TCH],
                                             start=(k == 0), stop=(k == KD - 1))
                        pss.append(ps)
                    return pss

                # P1a: xc own half + conv + silu
                for j in range(NJ):
                    xcp = p13.tile([P, L + D_CONV - 1], fp16, tag="xcp",
                                   name=f"xcp{j}", bufs=2)
                    nc.vector.memset(xcp[:, 0:D_CONV - 1], 0.0)
                    for t, ps in enumerate(in_proj_block(j, "lwx")):
                        nc.scalar.copy(
                            xcp[:, D_CONV - 1 + t * TCH:D_CONV - 1 + (t + 1) * TCH],
                            ps[:])
                    cacc = p13.tile([P, L], fp16, tag="cacc", name=f"cacc{j}",
                                    bufs=2)
                    nc.vector.tensor_scalar_mul(
                        cacc[:], xcp[:, 0:L],
                        convw_sb[:, j * D_CONV:j * D_CONV + 1])
                    for i in range(1, D_CONV):
                        nc.vector.scalar_tensor_tensor(
                            cacc[:], xcp[:, i:i + L],
                            convw_sb[:, j * D_CONV + i:j * D_CONV + i + 1],
                            cacc[:], Alu.mult, Alu.add)
                    nc.scalar.activation(xcs[j][:], cacc[:], Act.Silu,
                                         bias=convb_sb[:, j:j + 1])

                # P1b: x_proj partial over own half + pair AllReduce
                dblp = p13.tile([P, L], fp16, tag="dblp", name="dblp")
                for t in range(NT):
                    ps = psA.tile([P, TCH], f32, tag="ps", name=f"xp{t}")
                    for k in range(NJ):
                        nc.tensor.matmul(ps[:], xpw_sb[:, k * P:(k + 1) * P],
                                         xcs[k][:, t * TCH:(t + 1) * TCH],
                                         start=(k == 0), stop=(k == NJ - 1))
                    nc.scalar.copy(dblp[:, t * TCH:(t + 1) * TCH], ps[:])
                nc.sync.dma_start(dbl_in[:], dblp[:])
                nc.gpsimd.collective_compute(
                    "AllReduce", Alu.add, replica_groups=_PAIRS,
                    ins=[dbl_in[:]], outs=[dbl_out[:]])

                # P1c: z own half + silu (overlaps the AllReduce)
                for j in range(NJ):
                    for t, ps in enumerate(in_proj_block(NJ + j, "lwz")):
                        nc.scalar.activation(szs[j][:, t * TCH:(t + 1) * TCH],
                                             ps[:], Act.Silu)

                # P2: unpack AllReduce result (fp16 throughout)
                dt16 = p13.tile([DT_RANK, L], fp16, tag="dt16", name="dt16")
                nc.sync.dma_start(dt16[:], dbl_out[0:DT_RANK, :])
                nc.sync.dma_start(bcBC[:, 0:L],
                                  dbl_out[DT_RANK:DT_RANK + D_STATE, :])
                nc.sync.dma_start(bcBC[:, L:2 * L],
                                  dbl_out[DT_RANK + D_STATE:DT_RANK + 2 * D_STATE, :])

                # P2b: dt_proj + softplus -> delta (fp16); batch Exp then Ln
                spts = {}
                for j in range(NJ):
                    for t in range(NT):
                        ps = psA.tile([P, TCH], f32, tag="ps", name=f"dtp{j}_{t}")
                        nc.tensor.matmul(ps[:], dtw_sb[:, j * P:(j + 1) * P],
                                         dt16[:, t * TCH:(t + 1) * TCH],
                                         start=True, stop=True)
                        spt = p13.tile([P, TCH], fp16, tag=f"sp{j}_{t}",
                                       name=f"spt{j}_{t}")
                        nc.scalar.activation(spt[:], ps[:], Act.Exp,
                                             bias=dtb_sb[:, j:j + 1])
                        spts[(j, t)] = spt
                for j in range(NJ):
                    for t in range(NT):
                        nc.scalar.activation(deltas[j][:, t * TCH:(t + 1) * TCH],
                                             spts[(j, t)][:], Act.Ln, bias=1.0)

                # P3: wv, g0
                for j in range(NJ):
                    nc.vector.tensor_tensor(wvs[j][:], deltas[j][:], xcs[j][:],
                                            Alu.mult)
                    t1 = p13.tile([P, L], fp16, tag="g0t", name=f"g0t{j}", bufs=2)
                    nc.vector.tensor_scalar_mul(t1[:], xcs[j][:], D_sb[:, j:j + 1])
                    nc.vector.tensor_tensor(g0s[j][:], t1[:], szs[j][:], Alu.mult)

            # out_proj weights (2 MB fp16) load during the scan phase
            outw_sb = [cload(outw[m], (P, NJ * P), fp16, f"outw{m}")
                       for m in range(NJ)]

            # ================= P4: scan =================
            with tc.tile_pool(name="p4t", bufs=1) as p4t, \
                 tc.tile_pool(name="pscan", bufs=1, space="PSUM") as pscan:
                for hb in range(4):
                    js = [hb * 2, hb * 2 + 1]
                    yps = {j: pscan.tile([P, L], f32, tag=f"yps{hb % 2}_{j % 2}",
                                         name=f"yps{j}") for j in js}
                    for n in range(D_STATE):
                        bc = p4t.tile([P, 2 * L], fp16, tag="bc",
                                      name=f"bc{hb}_{n}", bufs=3)
                        nc.sync.dma_start(
                            bc[:],
                            bcBC[n:n + 1, :].partition_broadcast(P).squeeze(1))
                        for j in js:
                            a_t = p4t.tile([P, L], fp16, tag="a_t",
                                           name=f"a{j}_{n}", bufs=3)
                            nc.scalar.activation(
                                a_t[:], deltas[j][:], Act.Exp,
                                scale=A_sb[:, j * D_STATE + n:j * D_STATE + n + 1])
                            b_t = p4t.tile([P, L], fp16, tag="b_t",
                                           name=f"b{j}_{n}", bufs=2)
                            nc.vector.tensor_tensor(b_t[:], wvs[j][:], bc[:, 0:L],
                                                    Alu.mult)
                            h_t = p4t.tile([P, L], fp16, tag="h_t",
                                           name=f"h{j}_{n}", bufs=2)
                            nc.vector.tensor_tensor_scan(h_t[:], a_t[:], b_t[:],
                                                         0.0, Alu.mult, Alu.add)
                            prod = p4t.tile([P, L], fp16, tag="prod",
                                            name=f"p{j}_{n}", bufs=4)
                            eng = nc.gpsimd if n in GP_N else nc.vector
                            eng.tensor_tensor(prod[:], h_t[:], bc[:, L:2 * L],
                                              Alu.mult)
                            for t in range(NT):
                                sl = slice(t * TCH, (t + 1) * TCH)
                                nc.tensor.matmul(yps[j][:, sl], ident_sb[:],
                                                 prod[:, sl],
                                                 start=(n == 0),
                                                 stop=(n == D_STATE - 1))
                    for j in js:
                        yc = p4t.tile([P, L], fp16, tag="yc", name=f"yc{j}",
                                      bufs=2)
                        nc.scalar.copy(yc[:], yps[j][:])
                        ygt = p4t.tile([P, L], fp16, tag="ygt", name=f"ygt{j}",
                                       bufs=2)
                        nc.vector.tensor_tensor(ygt[:], yc[:], szs[j][:], Alu.mult)
                        nc.vector.tensor_tensor(ygs[j][:], ygt[:], g0s[j][:],
                                                Alu.add)

            # ================= P5: out_proj + RS =================
            with tc.tile_pool(name="p5t", bufs=1) as p5t, \
                 tc.tile_pool(name="psC", bufs=1, space="PSUM") as psC:
                for m in range(NJ):
                    ms = p5t.tile([P, L], fp16, tag="ms", name=f"ms{m}", bufs=2)
                    for t in range(NT):
                        ps = psC.tile([P, TCH], f32, tag="ps", name=f"op{m}_{t}", bufs=2)
                        for k in range(NJ):
                            nc.tensor.matmul(ps[:],
                                             outw_sb[m][:, k * P:(k + 1) * P],
                                             ygs[k][:, t * TCH:(t + 1) * TCH],
                                             start=(k == 0), stop=(k == NJ - 1))
                        nc.scalar.copy(ms[:, t * TCH:(t + 1) * TCH], ps[:])
                    msF = p5t.tile([P, L], fp16, tag="msF", name=f"msF{m}", bufs=2)
                    nc.scalar.activation(msF[:], ms[:], Act.Copy,
                                         scale=flags_sb[:, 0:1])
                    msB = p5t.tile([P, L], fp16, tag="msB", name=f"msB{m}", bufs=2)
                    nc.scalar.activation(msB[:], ms[:, ::-1], Act.Copy,
                                         scale=flags_sb[:, 1:2])
                    msb = p5t.tile([P, L], fp16, tag="msb", name=f"msb{m}", bufs=2)
                    nc.vector.tensor_tensor(msb[:], msF[:], msB[:], Alu.add)
                    dst = arqA if m < 4 else arqB
                    for q in range(4):
                        nc.sync.dma_start(
                            dst[q, (m % 4) * P:(m % 4 + 1) * P, :],
                            msb[:, q * LQ:(q + 1) * LQ])
                    if m == 3:
                        nc.gpsimd.collective_compute(
                            "ReduceScatter", Alu.add, replica_groups=_QUADS,
                            ins=[arqA[:]], outs=[rs_outA[:]])
                if True:
                    nc.gpsimd.collective_compute(
                        "ReduceScatter", Alu.add, replica_groups=_QUADS,
                        ins=[arqB[:]], outs=[rs_outB[:]])

                # ================= P6: merge + LN =================
                # prefetch first FFN weight tiles while the RS is in flight
                w1_pre = []
                for m in range(6):
                    lw = p5t.tile([P, KD * P], bf16, tag="w1", name=f"w1_{m}",
                                  bufs=6)
                    nc.sync.dma_start(lw[:], w1m[m])
                    w1_pre.append(lw)
                w2_pre = p5t.tile([P, NH1 * P], bf16, tag="w2", name="w2_0",
                                  bufs=2)
                nc.sync.dma_start(w2_pre[:], w2m[0])

                mos = [p5t.tile([P, LQ], fp16, tag=f"mo{j}", name=f"mo{j}")
                       for j in range(KD)]
                mu_ps = psC.tile([1, LQ], f32, tag="mu", name="mu_ps")
                e2_ps = psC.tile([1, LQ], f32, tag="e2", name="e2_ps")
                for j in range(KD):
                    rsrc = rs_outA if j < 4 else rs_outB
                    nc.sync.dma_start(mos[j][:], rsrc[(j % 4) * P:(j % 4 + 1) * P, :])
                    sq = p5t.tile([P, LQ], fp16, tag="sq", name=f"sq{j}", bufs=2)
                    nc.scalar.activation(sq[:], mos[j][:], Act.Square)
                    nc.tensor.matmul(mu_ps[:], ones_sb[:, 0:1], mos[j][:],
                                     start=(j == 0), stop=(j == KD - 1))
                    nc.tensor.matmul(e2_ps[:], ones_sb[:, 0:1], sq[:],
                                     start=(j == 0), stop=(j == KD - 1))
                mean = p5t.tile([1, LQ], f32, tag="mean", name="mean")
                nc.scalar.activation(mean[:], mu_ps[:], Act.Copy,
                                     scale=1.0 / D_MODEL)
                e2m = p5t.tile([1, LQ], f32, tag="e2m", name="e2m")
                nc.scalar.activation(e2m[:], e2_ps[:], Act.Copy,
                                     scale=1.0 / D_MODEL)
                m2 = p5t.tile([1, LQ], f32, tag="m2", name="m2")
                nc.vector.tensor_tensor(m2[:], mean[:], mean[:], Alu.mult)
                var = p5t.tile([1, LQ], f32, tag="var", name="var")
                nc.vector.tensor_tensor(var[:], e2m[:], m2[:], Alu.subtract)
                eps_sb = p5t.tile([1, 1], f32, tag="eps", name="eps_sb")
                nc.vector.memset(eps_sb[:], 1e-5)
                std = p5t.tile([1, LQ], f32, tag="std", name="std")
                nc.scalar.activation(std[:], var[:], Act.Sqrt, bias=eps_sb[:])
                rstd = p5t.tile([1, LQ], f32, tag="rstd", name="rstd")
                nc.vector.reciprocal(rstd[:], std[:])
                mean_h = p5t.tile([1, LQ], fp16, tag="mean_h", name="mean_h")
                nc.scalar.copy(mean_h[:], mean[:])
                rstd_h = p5t.tile([1, LQ], fp16, tag="rstd_h", name="rstd_h")
                nc.scalar.copy(rstd_h[:], rstd[:])
                mean_bc = psC.tile([P, LQ], f32, tag="mbc", name="mean_bc")
                nc.tensor.matmul(mean_bc[:], onesrow_sb[:], mean_h[:],
                                 start=True, stop=True)
                rstd_bc = psC.tile([P, LQ], f32, tag="rbc", name="rstd_bc")
                nc.tensor.matmul(rstd_bc[:], onesrow_sb[:], rstd_h[:],
                                 start=True, stop=True)

                xns = [p5t.tile([P, LQ], bf16, tag=f"xn{j}", name=f"xn{j}")
                       for j in range(KD)]
                for j in range(KD):
                    t1 = p5t.tile([P, LQ], f32, tag="lnt", name=f"lnt{j}", bufs=2)
                    nc.vector.tensor_tensor(t1[:], mos[j][:], mean_bc[:],
                                            Alu.subtract)
                    nc.vector.tensor_tensor(t1[:], t1[:], rstd_bc[:], Alu.mult)
                    nc.vector.tensor_scalar(xns[j][:], t1[:], lng_sb[:, j:j + 1],
                                            lnb_sb[:, j:j + 1], Alu.mult, Alu.add)

                # ================= P7: FFN =================
                with tc.tile_pool(name="ffh_pool", bufs=1) as ffh_pool:
                    ffhs = [ffh_pool.tile([P, LQ], bf16, tag=f"fh{m}",
                                          name=f"fh{m}") for m in range(NH1)]
                    for m in range(NH1):
                        if m < 6:
                            lw = w1_pre[m]
                        else:
                            lw = p5t.tile([P, KD * P], bf16, tag="w1",
                                          name=f"w1_{m}", bufs=6)
                            nc.sync.dma_start(lw[:], w1m[m])
                        ps = psC.tile([P, LQ], f32, tag="psf", name=f"f1{m}",
                                      bufs=2)
                        for k in range(KD):
                            nc.tensor.matmul(ps[:], lw[:, k * P:(k + 1) * P],
                                             xns[k][:],
                                             start=(k == 0), stop=(k == KD - 1))
                        nc.scalar.activation(ffhs[m][:], ps[:], Act.Gelu,
                                             bias=b1_sb[:, m:m + 1])

                    for m in range(KD):
                        if m == 0:
                            lw = w2_pre
                        else:
                            lw = p5t.tile([P, NH1 * P], bf16, tag="w2",
                                          name=f"w2_{m}", bufs=2)
                            nc.sync.dma_start(lw[:], w2m[m])
                        ps = psC.tile([P, LQ], f32, tag="psf", name=f"f2{m}",
                                      bufs=2)
                        for k in range(NH1):
                            nc.tensor.matmul(ps[:], lw[:, k * P:(k + 1) * P],
                                             ffhs[k][:],
                                             start=(k == 0), stop=(k == NH1 - 1))
                        ob = p5t.tile([P, LQ], f32, tag="ob", name=f"ob{m}",
                                      bufs=2)
                        nc.vector.tensor_scalar_add(ob[:], ps[:],
                                                    b2_sb[:, m:m + 1])
                        nc.sync.dma_start(out_m[m * P:(m + 1) * P, :], ob[:])

    nc.compile()
    return nc


def _prep_inputs(inputs):
    """Per-core input dicts. Core c: sequence s=c//2 (s>=2 => time-flipped x),
    d_inner half = c%2."""
    import ml_dtypes
    bf = ml_dtypes.bfloat16
    fh = np.float16

    x = np.asarray(inputs["x"], dtype=np.float32)
    in_proj_w = np.asarray(inputs["in_proj_w"], dtype=np.float32)
    conv_w = np.asarray(inputs["conv_w"], dtype=np.float32)
    conv_b = np.asarray(inputs["conv_b"], dtype=np.float32)
    x_proj_w = np.asarray(inputs["x_proj_w"], dtype=np.float32)
    dt_proj_w = np.asarray(inputs["dt_proj_w"], dtype=np.float32)
    dt_proj_b = np.asarray(inputs["dt_proj_b"], dtype=np.float32)
    A = -np.exp(np.asarray(inputs["A_log"], dtype=np.float32))
    Dp = np.asarray(inputs["D"], dtype=np.float32)
    out_proj_w = np.asarray(inputs["out_proj_w"], dtype=np.float32)
    ln_g = np.asarray(inputs["ln_g"], dtype=np.float32)
    ln_b = np.asarray(inputs["ln_b"], dtype=np.float32)
    ff_w1 = np.asarray(inputs["ff_w1"], dtype=np.float32)
    ff_b1 = np.asarray(inputs["ff_b1"], dtype=np.float32)
    ff_w2 = np.asarray(inputs["ff_w2"], dtype=np.float32)
    ff_b2 = np.asarray(inputs["ff_b2"], dtype=np.float32)

    def cols(v):  # (N,) -> (P, N//P)
        return np.ascontiguousarray(v.reshape(-1, P).T)

    def wblocks(w, dt):  # (K, M) -> (M//P, P(kpart), K//P, P(m))
        K, M = w.shape
        r = w.reshape(K // P, P, M // P, P).transpose(2, 1, 0, 3)
        return np.ascontiguousarray(r.astype(dt))

    # shared across cores
    w1_t = wblocks(ff_w1, bf)                      # (32, P, 8, P)
    w2_t = wblocks(ff_w2, bf)                      # (8, P, 32, P)
    lngc = cols(ln_g)
    lnbc = cols(ln_b)
    b1c = cols(ff_b1)
    b2c = cols(ff_b2)
    identh = np.eye(P, dtype=fh)
    ones_h = np.ones((P, 2), fh)
    onesrow_h = np.ones((1, P), fh)

    in_maps = []
    for c in range(8):
        s, half = c // 2, c % 2
        xb = x[s] if s < 2 else x[s - 2][::-1]
        own = np.arange(half * HALF, (half + 1) * HALF)

        wxc = in_proj_w[:, own]                    # (1024, 1024)
        wz = in_proj_w[:, D_INNER + own]
        w_in = np.concatenate(
            [wblocks(wxc, bf), wblocks(wz, bf)], axis=0)   # (16, P, 8, P)

        cw = conv_w[own]
        convw_cols = np.ascontiguousarray(
            cw.reshape(NJ, P, D_CONV).transpose(1, 0, 2).reshape(P, NJ * D_CONV))

        xp = np.concatenate(
            [x_proj_w[own], np.zeros((HALF, P - DT_RANK - 2 * D_STATE),
                                     np.float32)], axis=1)  # (1024, 128)
        xpw_t = wblocks(xp, fh)[0]                 # (P, 8, P)

        dtw_t = np.ascontiguousarray(
            dt_proj_w[:, own].reshape(DT_RANK, NJ, P).astype(fh))

        A_colsv = np.ascontiguousarray(
            A[own].reshape(NJ, P, D_STATE).transpose(1, 0, 2).reshape(
                P, NJ * D_STATE))

        outw_t = wblocks(out_proj_w[own], fh)      # (8, P, 8, P)

        fwd = 1.0 if s < 2 else 0.0
        flags = np.concatenate([np.full((P, 1), fwd, np.float32),
                                np.full((P, 1), 1.0 - fwd, np.float32)], axis=1)

        in_maps.append({
            "xT": np.ascontiguousarray(xb.T).astype(bf),
            "w_in": w_in,
            "convw_cols": convw_cols,
            "convb_cols": cols(conv_b[own]),
            "xpw": xpw_t,
            "dtw": dtw_t,
            "dtb_cols": cols(dt_proj_b[own]),
            "A_cols": A_colsv,
            "D_cols": cols(Dp[own]),
            "outw": outw_t,
            "lng_cols": lngc,
            "lnb_cols": lnbc,
            "w1m": w1_t,
            "b1_cols": b1c,
            "w2m": w2_t,
            "b2_cols": b2c,
            "identh": identh,
            "ones_h": ones_h,
            "onesrow_h": onesrow_h,
            "flags": flags,
        })
    return in_maps


_NC_CACHE = {}


def _get_nc():
    if "nc" not in _NC_CACHE:
        _NC_CACHE["nc"] = _build_nc()
    return _NC_CACHE["nc"]


def run(inputs, trace=False):
    _install_ntff_hook_shim()
    from concourse import bass_utils
    nc = _get_nc()
    in_maps = _prep_inputs(inputs)
    res = bass_utils.run_bass_kernel_spmd(nc, in_maps, core_ids=list(range(8)),
                                          trace=trace)
    # core at quad-rank q holds t-columns [q*256, (q+1)*256) of its batch
    full = np.zeros((2, D_MODEL, L), np.float32)
    for c in range(8):
        b = 0 if c in _QUADS[0] else 1
        q = _QUADS[b].index(c)
        full[b, :, q * LQ:(q + 1) * LQ] = res.results[c]["out_m"]
    out = np.ascontiguousarray(full.transpose(0, 2, 1))
    return out, res


def kernel(**inputs):
    out, _ = run(inputs, trace=False)
    return out
